# revision 1
# baseline (speedup 1.0000x reference)
"""Bass/Trainium2 kernel for BERT-style masked attention (B=1, S=4096, HID=1024, H=16).

Strategy: tensor-parallel over heads across 8 NeuronCores (2 heads/core).
Each core computes q/k/v projections for its 128 output columns from the
full (host-pretransposed) hidden states, runs masked softmax attention for
its 2 heads fully on-chip (flash-style, scores never hit DRAM), and writes
its [4096, 128] slice of the context. Host concatenates slices.
"""

import numpy as np
from contextlib import ExitStack

import concourse.bass as bass
import concourse.tile as tile
from concourse import bacc, mybir
from concourse.bass_utils import run_bass_kernel_spmd
from concourse.masks import make_identity

f32 = mybir.dt.float32
f32r = mybir.dt.float32r
AF = mybir.ActivationFunctionType

S = 4096
HID = 1024
D2 = 128          # per-core output columns (2 heads x 64)
NCH = HID // 128  # 8 hid chunks
NSB = S // 512    # 8 sequence blocks for projections
NKC = S // 128    # 32 key chunks
NQT = S // 512    # 8 query tiles
SCALE = 64 ** -0.5
NEG = -1e30


def _emit(nc, tc, XT, WQ, WK, WV, BQ, BK, BV, MB, ONESM, OUT):
    with ExitStack() as top:
        const = top.enter_context(tc.tile_pool(name="const", bufs=1))
        big = top.enter_context(tc.tile_pool(name="big", bufs=1))

        ident = const.tile([128, 128], f32)
        make_identity(nc, ident)

        wq = const.tile([128, NCH, 128], f32r)
        wk = const.tile([128, NCH, 128], f32r)
        wv = const.tile([128, NCH, 128], f32r)
        nc.sync.dma_start(out=wq, in_=WQ.rearrange("(c p) d -> p c d", p=128).bitcast(f32r))
        nc.sync.dma_start(out=wk, in_=WK.rearrange("(c p) d -> p c d", p=128).bitcast(f32r))
        nc.sync.dma_start(out=wv, in_=WV.rearrange("(c p) d -> p c d", p=128).bitcast(f32r))

        bq = const.tile([128, 1], f32)
        bk = const.tile([128, 1], f32)
        bv = const.tile([128, 1], f32)
        nc.sync.dma_start(out=bq, in_=BQ.unsqueeze(1))
        nc.sync.dma_start(out=bk, in_=BK.unsqueeze(1))
        nc.sync.dma_start(out=bv, in_=BV.unsqueeze(1))

        mb = const.tile([128, NKC], f32)
        nc.sync.dma_start(out=mb, in_=MB)

        qT = big.tile([128, S], f32r)   # [d2, s] queries (both heads stacked)
        kT = big.tile([128, S], f32r)   # [d2, s] keys
        vT = big.tile([128, S], f32)    # [d2, s] values (pre-transpose)
        v1 = big.tile([128, 2, NKC, 65], f32r)  # [k, head, chunk, d|1]

        # ones column of v1 via broadcast DMA
        ones_in = bass.AP(
            tensor=ONESM.tensor, offset=ONESM.offset,
            ap=[[0, 128], [NKC, 2], [1, NKC], [0, 1]],
        ).bitcast(f32r)
        nc.sync.dma_start(out=v1[:, :, :, 64:65], in_=ones_in)

        # ---- projections: qT/kT/vT = W.T @ xT (accumulate over hid chunks)
        with tc.tile_pool(name="xwp", bufs=2) as xwp, \
             tc.tile_pool(name="pp", bufs=2, space="PSUM") as pp:
            for sb in range(NSB):
                sl = slice(sb * 512, (sb + 1) * 512)
                xw = xwp.tile([128, NCH, 512], f32r, tag="xw")
                for c in range(NCH):
                    nc.sync.dma_start(
                        out=xw[:, c, :],
                        in_=XT[c * 128:(c + 1) * 128, sl].bitcast(f32r))
                pq = pp.tile([128, 512], f32, tag="pq")
                for c in range(NCH):
                    nc.tensor.matmul(pq, wq[:, c, :], xw[:, c, :],
                                     start=(c == 0), stop=(c == NCH - 1))
                nc.vector.tensor_scalar_add(qT[:, sl], pq, bq)
                pk = pp.tile([128, 512], f32, tag="pk")
                for c in range(NCH):
                    nc.tensor.matmul(pk, wk[:, c, :], xw[:, c, :],
                                     start=(c == 0), stop=(c == NCH - 1))
                nc.vector.tensor_scalar_add(kT[:, sl], pk, bk)
                pv = pp.tile([128, 512], f32, tag="pv")
                for c in range(NCH):
                    nc.tensor.matmul(pv, wv[:, c, :], xw[:, c, :],
                                     start=(c == 0), stop=(c == NCH - 1))
                nc.vector.tensor_scalar_add(vT[:, sl], pv, bv)

        # ---- v1 = vT transposed into [k, d] chunks (+ ones col already set)
        with tc.tile_pool(name="ptp", bufs=3, space="PSUM") as ptp:
            for kc in range(NKC):
                pt = ptp.tile([128, 128], f32, tag="pt")
                nc.tensor.transpose(pt, vT[:, kc * 128:(kc + 1) * 128], ident)
                nc.vector.tensor_copy(v1[:, 0, kc, 0:64], pt[:, 0:64])
                nc.vector.tensor_copy(v1[:, 1, kc, 0:64], pt[:, 64:128])

        # ---- attention (flash-style, scores transposed [k, q])
        with tc.tile_pool(name="pss", bufs=3, space="PSUM") as pss, \
             tc.tile_pool(name="psc", bufs=2, space="PSUM") as psc, \
             tc.tile_pool(name="pst", bufs=2, space="PSUM") as pst, \
             tc.tile_pool(name="ep", bufs=3) as ep, \
             tc.tile_pool(name="cp", bufs=4) as cp, \
             tc.tile_pool(name="op", bufs=3) as op, \
             tc.tile_pool(name="lp", bufs=6) as lp:
            for qt in range(NQT):
                qsl = slice(qt * 512, (qt + 1) * 512)
                cts = []
                for h in range(2):
                    hsl = slice(h * 64, (h + 1) * 64)
                    pc = psc.tile([65, 512], f32, tag="pc")
                    e_prev = None
                    for kc in range(NKC):
                        ss = pss.tile([128, 512], f32, tag="ss")
                        nc.tensor.matmul(
                            ss, kT[hsl, kc * 128:(kc + 1) * 128], qT[hsl, qsl],
                            start=True, stop=True)
                        et = ep.tile([128, 512], f32r, tag="et")
                        nc.scalar.activation(et, ss, AF.Exp,
                                             bias=mb[:, kc:kc + 1], scale=SCALE)
                        if e_prev is not None:
                            nc.tensor.matmul(pc, v1[:, h, kc - 1, :], e_prev,
                                             start=(kc == 1), stop=False)
                        e_prev = et
                    nc.tensor.matmul(pc, v1[:, h, NKC - 1, :], e_prev,
                                     start=False, stop=True)
                    ct = cp.tile([65, 512], f32, tag="ct")
                    nc.vector.tensor_copy(ct, pc)
                    cts.append(ct)
                for qb in range(4):
                    pt2 = pst.tile([128, 130], f32, tag="pt2")
                    bsl = slice(qb * 128, (qb + 1) * 128)
                    nc.tensor.transpose(pt2[:, 0:65], cts[0][:, bsl], ident[0:65, 0:65])
                    nc.tensor.transpose(pt2[:, 65:130], cts[1][:, bsl], ident[0:65, 0:65])
                    li0 = lp.tile([128, 1], f32, tag="li")
                    li1 = lp.tile([128, 1], f32, tag="li")
                    nc.vector.reciprocal(li0, pt2[:, 64:65])
                    nc.vector.reciprocal(li1, pt2[:, 129:130])
                    ob = op.tile([128, 128], f32, tag="ob")
                    nc.vector.tensor_scalar_mul(ob[:, 0:64], pt2[:, 0:64], li0)
                    nc.vector.tensor_scalar_mul(ob[:, 64:128], pt2[:, 65:129], li1)
                    r = qt * 4 + qb
                    nc.sync.dma_start(out=OUT[r * 128:(r + 1) * 128, :], in_=ob)


_NC = None


def _build():
    global _NC
    if _NC is not None:
        return _NC
    nc = bacc.Bacc("TRN2", target_bir_lowering=False, debug=False)
    XT = nc.dram_tensor("XT", [HID, S], f32, kind="ExternalInput").ap()
    WQ = nc.dram_tensor("WQ", [HID, D2], f32, kind="ExternalInput").ap()
    WK = nc.dram_tensor("WK", [HID, D2], f32, kind="ExternalInput").ap()
    WV = nc.dram_tensor("WV", [HID, D2], f32, kind="ExternalInput").ap()
    BQ = nc.dram_tensor("BQ", [D2], f32, kind="ExternalInput").ap()
    BK = nc.dram_tensor("BK", [D2], f32, kind="ExternalInput").ap()
    BV = nc.dram_tensor("BV", [D2], f32, kind="ExternalInput").ap()
    MB = nc.dram_tensor("MB", [128, NKC], f32, kind="ExternalInput").ap()
    ONESM = nc.dram_tensor("ONESM", [2, NKC], f32, kind="ExternalInput").ap()
    OUT = nc.dram_tensor("OUT", [S, D2], f32, kind="ExternalOutput").ap()
    with tile.TileContext(nc) as tc:
        _emit(nc, tc, XT, WQ, WK, WV, BQ, BK, BV, MB, ONESM, OUT)
    nc.compile()
    _NC = nc
    return nc


def make_in_maps(hidden_states, attention_mask, Wq, bq, Wk, bk, Wv, bv):
    x = np.asarray(hidden_states, dtype=np.float32).reshape(S, HID)
    xT = np.ascontiguousarray(x.T)
    mask = np.asarray(attention_mask).reshape(S)
    mbias = np.where(mask.astype(bool), np.float32(0.0), np.float32(NEG))
    MBn = np.ascontiguousarray(mbias.reshape(NKC, 128).T.astype(np.float32))
    ones = np.ones((2, NKC), np.float32)
    Wq = np.asarray(Wq, np.float32)
    Wk = np.asarray(Wk, np.float32)
    Wv = np.asarray(Wv, np.float32)
    bq = np.asarray(bq, np.float32)
    bk = np.asarray(bk, np.float32)
    bv = np.asarray(bv, np.float32)
    in_maps = []
    for c in range(8):
        sl = slice(D2 * c, D2 * (c + 1))
        in_maps.append({
            "XT": xT, "MB": MBn, "ONESM": ones,
            "WQ": np.ascontiguousarray(Wq[:, sl]),
            "WK": np.ascontiguousarray(Wk[:, sl]),
            "WV": np.ascontiguousarray(Wv[:, sl]),
            "BQ": np.ascontiguousarray(bq[sl]),
            "BK": np.ascontiguousarray(bk[sl]),
            "BV": np.ascontiguousarray(bv[sl]),
        })
    return in_maps


def kernel(hidden_states, attention_mask, Wq, bq, Wk, bk, Wv, bv):
    nc = _build()
    in_maps = make_in_maps(hidden_states, attention_mask, Wq, bq, Wk, bk, Wv, bv)
    res = run_bass_kernel_spmd(nc, in_maps, list(range(8)))
    out = np.concatenate([res.results[c]["OUT"] for c in range(8)], axis=1)
    return (out.reshape(1, S, HID),)


# revision 5
# speedup vs baseline: 2.1724x; 2.1724x over previous
"""Bass/Trainium2 kernel for BERT-style masked attention (B=1, S=4096, HID=1024, H=16).

Strategy: tensor-parallel over heads across 8 NeuronCores (2 heads/core).
Each core computes q/k/v projections for its 128 output columns from the
full (host-pretransposed) hidden states, runs masked softmax attention for
its 2 heads fully on-chip (flash-style, scores never hit DRAM), and writes
its [4096, 128] slice of the context. Host concatenates slices.

The key mask is key-only (same for every query/head), so masked key
positions are compacted away host-side: k/v projections and the attention
inner loop run only over the ~(S/2) surviving key positions.
"""

import numpy as np
from contextlib import ExitStack

import concourse.bass as bass
import concourse.tile as tile
from concourse import bacc, mybir
from concourse.bass_utils import run_bass_kernel_spmd
from concourse.masks import make_identity

f32 = mybir.dt.float32
f32r = mybir.dt.float32r
AF = mybir.ActivationFunctionType

S = 4096
HID = 1024
D2 = 128          # per-core output columns (2 heads x 64)
NCH = HID // 128  # 8 hid chunks
NSB = S // 512    # 8 sequence blocks for q projections
NQT = S // 512    # 8 query tiles
SCALE = 64 ** -0.5
NEG = -1e30
GRP = 2           # key chunks per attention group


def _emit(nc, tc, aps, nkb, nkca):
    """nkb: # 512-wide key blocks for k/v projections (SKP = 512*nkb).
    nkca: # 128-wide key chunks the attention loop visits (<= 4*nkb)."""
    XT, XTKV, WQ, WK, WV, BQ, BK, BV, MB, OUT = aps
    skp = 512 * nkb
    with ExitStack() as top:
        const = top.enter_context(tc.tile_pool(name="const", bufs=1))
        big = top.enter_context(tc.tile_pool(name="big", bufs=1))

        ident = const.tile([128, 128], f32)
        make_identity(nc, ident)

        wq = const.tile([128, NCH, 128], f32r)
        wk = const.tile([128, NCH, 128], f32r)
        wv = const.tile([128, NCH, 128], f32r)
        nc.gpsimd.dma_start(out=wq, in_=WQ.rearrange("(c p) d -> p c d", p=128).bitcast(f32r))
        nc.gpsimd.dma_start(out=wk, in_=WK.rearrange("(c p) d -> p c d", p=128).bitcast(f32r))
        nc.gpsimd.dma_start(out=wv, in_=WV.rearrange("(c p) d -> p c d", p=128).bitcast(f32r))

        bq = const.tile([128, 1], f32)
        bk = const.tile([128, 1], f32)
        bv = const.tile([128, 1], f32)
        nc.gpsimd.dma_start(out=bq, in_=BQ.unsqueeze(1))
        nc.gpsimd.dma_start(out=bk, in_=BK.unsqueeze(1))
        nc.gpsimd.dma_start(out=bv, in_=BV.unsqueeze(1))

        mb = const.tile([128, nkca], f32)
        nc.gpsimd.dma_start(out=mb, in_=MB)

        qT = big.tile([128, S], f32r)     # [d2, s] queries (both heads stacked)
        kT = big.tile([128, skp], f32r)   # [d2, sk] keys (compacted)
        vT = big.tile([128, skp], f32)    # [d2, sk] values (pre-transpose)
        v1 = big.tile([128, 2, nkca, 65], f32r)  # [k, head, chunk, d|1]
        ones_c = const.tile([128, 1], f32)
        nc.vector.memset(ones_c, 1.0)
        nc.vector.tensor_copy(v1[:, 0, :, 64:65], ones_c.to_broadcast((128, nkca, 1)))
        nc.vector.tensor_copy(v1[:, 1, :, 64:65], ones_c.to_broadcast((128, nkca, 1)))

        # ---- projections (accumulate over hid chunks, f32r full-rate N=512)
        with tc.tile_pool(name="xwp", bufs=3) as xwp, \
             tc.tile_pool(name="pp", bufs=2, space="PSUM") as pp:
            for sb in range(NSB):
                sl = slice(sb * 512, (sb + 1) * 512)
                xw = xwp.tile([128, NCH, 512], f32r, tag="xw")
                for c in range(NCH):
                    nc.sync.dma_start(
                        out=xw[:, c, :],
                        in_=XT[c * 128:(c + 1) * 128, sl].bitcast(f32r))
                pq = pp.tile([128, 512], f32, tag="pq")
                for c in range(NCH):
                    nc.tensor.matmul(pq, wq[:, c, :], xw[:, c, :],
                                     start=(c == 0), stop=(c == NCH - 1))
                nc.vector.tensor_scalar_add(qT[:, sl], pq, bq)
            for kb in range(nkb):
                sl = slice(kb * 512, (kb + 1) * 512)
                xw = xwp.tile([128, NCH, 512], f32r, tag="xw")
                for c in range(NCH):
                    nc.sync.dma_start(
                        out=xw[:, c, :],
                        in_=XTKV[c * 128:(c + 1) * 128, sl].bitcast(f32r))
                pk = pp.tile([128, 512], f32, tag="pq")
                for c in range(NCH):
                    nc.tensor.matmul(pk, wk[:, c, :], xw[:, c, :],
                                     start=(c == 0), stop=(c == NCH - 1))
                nc.vector.tensor_scalar_add(kT[:, sl], pk, bk)
                pv = pp.tile([128, 512], f32, tag="pq")
                for c in range(NCH):
                    nc.tensor.matmul(pv, wv[:, c, :], xw[:, c, :],
                                     start=(c == 0), stop=(c == NCH - 1))
                nc.vector.tensor_scalar_add(vT[:, sl], pv, bv)

        # ---- v1 = vT transposed into [k, d] chunks (+ ones col via memset)
        with tc.tile_pool(name="ptp", bufs=3, space="PSUM") as ptp:
            for kc in range(nkca):
                pt = ptp.tile([128, 128], f32, tag="pt")
                nc.tensor.transpose(pt, vT[:, kc * 128:(kc + 1) * 128], ident)
                nc.vector.tensor_copy(v1[:, 0, kc, 0:64], pt[:, 0:64])
                nc.vector.tensor_copy(v1[:, 1, kc, 0:64], pt[:, 64:128])

        # ---- attention (flash-style, scores transposed [k, q])
        ngr = (nkca + GRP - 1) // GRP
        groups = [(g * GRP, min(GRP, nkca - g * GRP)) for g in range(ngr)]
        with tc.tile_pool(name="pss", bufs=2, space="PSUM") as pss, \
             tc.tile_pool(name="psc", bufs=2, space="PSUM") as psc, \
             tc.tile_pool(name="pst", bufs=2, space="PSUM") as pst, \
             tc.tile_pool(name="ep", bufs=2) as ep, \
             tc.tile_pool(name="cp", bufs=4) as cp, \
             tc.tile_pool(name="op", bufs=3) as op, \
             tc.tile_pool(name="lp", bufs=6) as lp:
            for qt in range(NQT):
                qsl = slice(qt * 512, (qt + 1) * 512)
                cts = []
                for h in range(2):
                    hsl = slice(h * 64, (h + 1) * 64)
                    pc = psc.tile([65, 512], f32, tag="pc")
                    prev = None  # (eT tile, base chunk, group size)
                    for (k0, gs) in groups:
                        ss = pss.tile([128, GRP, 512], f32, tag="ss")
                        for j in range(gs):
                            kc = k0 + j
                            nc.tensor.matmul(
                                ss[:, j, :],
                                kT[hsl, kc * 128:(kc + 1) * 128], qT[hsl, qsl],
                                start=True, stop=True)
                        et = ep.tile([128, GRP, 512], f32r, tag="et")
                        if k0 + gs < nkca:
                            # all-real chunks: no mask needed (compacted keys)
                            nc.scalar.activation(
                                et[:, 0:gs, :], ss[:, 0:gs, :], AF.Exp,
                                bias=0.0, scale=SCALE)
                        else:
                            # final group: last chunk carries pad-slot mask
                            if gs > 1:
                                nc.scalar.activation(
                                    et[:, 0:gs - 1, :], ss[:, 0:gs - 1, :],
                                    AF.Exp, bias=0.0, scale=SCALE)
                            nc.scalar.activation(
                                et[:, gs - 1, :], ss[:, gs - 1, :], AF.Exp,
                                bias=mb[:, nkca - 1:nkca], scale=SCALE)
                        if prev is not None:
                            pk0, pgs, pet = prev
                            for j in range(pgs):
                                nc.tensor.matmul(pc, v1[:, h, pk0 + j, :],
                                                 pet[:, j, :],
                                                 start=(pk0 + j == 0), stop=False)
                        prev = (k0, gs, et)
                    pk0, pgs, pet = prev
                    for j in range(pgs):
                        nc.tensor.matmul(pc, v1[:, h, pk0 + j, :], pet[:, j, :],
                                         start=(pk0 + j == 0), stop=(j == pgs - 1))
                    ct = cp.tile([65, 512], f32, tag="ct")
                    nc.vector.tensor_copy(ct, pc)
                    cts.append(ct)
                for qb in range(4):
                    pt2 = pst.tile([128, 130], f32, tag="pt2")
                    bsl = slice(qb * 128, (qb + 1) * 128)
                    nc.tensor.transpose(pt2[:, 0:65], cts[0][:, bsl], ident[0:65, 0:65])
                    nc.tensor.transpose(pt2[:, 65:130], cts[1][:, bsl], ident[0:65, 0:65])
                    li0 = lp.tile([128, 1], f32, tag="li")
                    li1 = lp.tile([128, 1], f32, tag="li")
                    nc.vector.reciprocal(li0, pt2[:, 64:65])
                    nc.vector.reciprocal(li1, pt2[:, 129:130])
                    ob = op.tile([128, 128], f32, tag="ob")
                    nc.vector.tensor_scalar_mul(ob[:, 0:64], pt2[:, 0:64], li0)
                    nc.vector.tensor_scalar_mul(ob[:, 64:128], pt2[:, 65:129], li1)
                    r = qt * 4 + qb
                    nc.sync.dma_start(out=OUT[r * 128:(r + 1) * 128, :], in_=ob)


_NC = {}


def _build(nkb, nkca):
    key = (nkb, nkca)
    if key in _NC:
        return _NC[key]
    nc = bacc.Bacc("TRN2", target_bir_lowering=False, debug=False)
    skp = 512 * nkb
    XT = nc.dram_tensor("XT", [HID, S], f32, kind="ExternalInput").ap()
    XTKV = nc.dram_tensor("XTKV", [HID, skp], f32, kind="ExternalInput").ap()
    WQ = nc.dram_tensor("WQ", [HID, D2], f32, kind="ExternalInput").ap()
    WK = nc.dram_tensor("WK", [HID, D2], f32, kind="ExternalInput").ap()
    WV = nc.dram_tensor("WV", [HID, D2], f32, kind="ExternalInput").ap()
    BQ = nc.dram_tensor("BQ", [D2], f32, kind="ExternalInput").ap()
    BK = nc.dram_tensor("BK", [D2], f32, kind="ExternalInput").ap()
    BV = nc.dram_tensor("BV", [D2], f32, kind="ExternalInput").ap()
    MB = nc.dram_tensor("MB", [128, nkca], f32, kind="ExternalInput").ap()
    OUT = nc.dram_tensor("OUT", [S, D2], f32, kind="ExternalOutput").ap()
    with tile.TileContext(nc) as tc:
        _emit(nc, tc, (XT, XTKV, WQ, WK, WV, BQ, BK, BV, MB, OUT), nkb, nkca)
    nc.compile()
    _NC[key] = nc
    return nc


def make_in_maps(hidden_states, attention_mask, Wq, bq, Wk, bk, Wv, bv):
    x = np.asarray(hidden_states, dtype=np.float32).reshape(S, HID)
    xT = np.ascontiguousarray(x.T)
    mask = np.asarray(attention_mask).reshape(S).astype(bool)
    idx = np.nonzero(mask)[0]
    m = len(idx)
    nkca = max(1, (m + 127) // 128)
    nkb = max(1, (nkca * 128 + 511) // 512)
    skp = nkb * 512
    # pad with position 0 (values are finite; pad slots masked to -inf below)
    idx_p = np.zeros(skp, np.int64)
    idx_p[:m] = idx
    xTkv = np.ascontiguousarray(xT[:, idx_p])
    mbias = np.full(nkca * 128, np.float32(NEG), np.float32)
    mbias[:m] = 0.0
    MBn = np.ascontiguousarray(mbias.reshape(nkca, 128).T)
    Wq = np.asarray(Wq, np.float32)
    Wk = np.asarray(Wk, np.float32)
    Wv = np.asarray(Wv, np.float32)
    bq = np.asarray(bq, np.float32)
    bk = np.asarray(bk, np.float32)
    bv = np.asarray(bv, np.float32)
    in_maps = []
    for c in range(8):
        sl = slice(D2 * c, D2 * (c + 1))
        in_maps.append({
            "XT": xT, "XTKV": xTkv, "MB": MBn,
            "WQ": np.ascontiguousarray(Wq[:, sl]),
            "WK": np.ascontiguousarray(Wk[:, sl]),
            "WV": np.ascontiguousarray(Wv[:, sl]),
            "BQ": np.ascontiguousarray(bq[sl]),
            "BK": np.ascontiguousarray(bk[sl]),
            "BV": np.ascontiguousarray(bv[sl]),
        })
    return in_maps, nkb, nkca


def kernel(hidden_states, attention_mask, Wq, bq, Wk, bk, Wv, bv):
    in_maps, nkb, nkca = make_in_maps(
        hidden_states, attention_mask, Wq, bq, Wk, bk, Wv, bv)
    nc = _build(nkb, nkca)
    res = run_bass_kernel_spmd(nc, in_maps, list(range(8)))
    out = np.concatenate([res.results[c]["OUT"] for c in range(8)], axis=1)
    return (out.reshape(1, S, HID),)


# revision 6
# speedup vs baseline: 3.0534x; 1.4056x over previous
"""Bass/Trainium2 kernel for BERT-style masked attention (B=1, S=4096, HID=1024, H=16).

Strategy: tensor-parallel over heads across 8 NeuronCores (2 heads/core).
Each core computes q/k/v projections for its 128 output columns from the
full (host-pretransposed) hidden states, runs masked softmax attention for
its 2 heads fully on-chip (flash-style, scores never hit DRAM), and writes
its [4096, 128] slice of the context. Host concatenates slices.

The key mask is key-only (same for every query/head), so masked key
positions are compacted away host-side: k/v projections and the attention
inner loop run only over the ~(S/2) surviving key positions.
"""

import numpy as np
from contextlib import ExitStack

import concourse.bass as bass
import concourse.tile as tile
from concourse import bacc, mybir
from concourse.bass_utils import run_bass_kernel_spmd
from concourse.masks import make_identity

f32 = mybir.dt.float32
f32r = mybir.dt.float32r
AF = mybir.ActivationFunctionType

S = 4096
HID = 1024
D2 = 128          # per-core output columns (2 heads x 64)
NCH = HID // 128  # 8 hid chunks
NSB = S // 512    # 8 sequence blocks for q projections
NQT = S // 512    # 8 query tiles
SCALE = 64 ** -0.5
NEG = -1e30
GRP = 2           # key chunks per attention group


def _emit(nc, tc, aps, nkb, nkca):
    """nkb: # 512-wide key blocks for k/v projections (SKP = 512*nkb).
    nkca: # 128-wide key chunks the attention loop visits (<= 4*nkb)."""
    XT, XTKV, WQ, WK, WV, BQ, BK, BV, MB, OUT = aps
    skp = 512 * nkb
    with ExitStack() as top:
        const = top.enter_context(tc.tile_pool(name="const", bufs=1))
        big = top.enter_context(tc.tile_pool(name="big", bufs=1))

        ident = const.tile([128, 128], f32)
        make_identity(nc, ident)

        wq = const.tile([128, NCH, 128], f32r)
        wk = const.tile([128, NCH, 128], f32r)
        wv = const.tile([128, NCH, 128], f32r)
        nc.gpsimd.dma_start(out=wq, in_=WQ.rearrange("(c p) d -> p c d", p=128).bitcast(f32r))
        nc.gpsimd.dma_start(out=wk, in_=WK.rearrange("(c p) d -> p c d", p=128).bitcast(f32r))
        nc.gpsimd.dma_start(out=wv, in_=WV.rearrange("(c p) d -> p c d", p=128).bitcast(f32r))

        bq = const.tile([128, 1], f32)
        bk = const.tile([128, 1], f32)
        bv = const.tile([128, 1], f32)
        nc.gpsimd.dma_start(out=bq, in_=BQ.unsqueeze(1))
        nc.gpsimd.dma_start(out=bk, in_=BK.unsqueeze(1))
        nc.gpsimd.dma_start(out=bv, in_=BV.unsqueeze(1))

        mb = const.tile([128, nkca], f32)
        nc.gpsimd.dma_start(out=mb, in_=MB)

        qT = big.tile([128, S], f32r)     # [d2, s] queries (both heads stacked)
        kT = big.tile([128, skp], f32r)   # [d2, sk] keys (compacted)
        vT = big.tile([128, skp], f32)    # [d2, sk] values (pre-transpose)
        v1 = big.tile([128, 2, nkca, 65], f32r)  # [k, head, chunk, d|1]
        ones_c = const.tile([128, 1], f32)
        nc.vector.memset(ones_c, 1.0)
        nc.vector.tensor_copy(v1[:, 0, :, 64:65], ones_c.to_broadcast((128, nkca, 1)))
        nc.vector.tensor_copy(v1[:, 1, :, 64:65], ones_c.to_broadcast((128, nkca, 1)))

        # ---- projections (accumulate over hid chunks, f32r full-rate N=512)
        with tc.tile_pool(name="xwp", bufs=3) as xwp, \
             tc.tile_pool(name="pp", bufs=2, space="PSUM") as pp:
            for sb in range(NSB):
                sl = slice(sb * 512, (sb + 1) * 512)
                xw = xwp.tile([128, NCH, 512], f32r, tag="xw")
                for c in range(NCH):
                    nc.sync.dma_start(
                        out=xw[:, c, :],
                        in_=XT[c * 128:(c + 1) * 128, sl].bitcast(f32r))
                pq = pp.tile([128, 512], f32, tag="pq")
                for c in range(NCH):
                    nc.tensor.matmul(pq, wq[:, c, :], xw[:, c, :],
                                     start=(c == 0), stop=(c == NCH - 1))
                nc.vector.tensor_scalar_add(qT[:, sl], pq, bq)
            for kb in range(nkb):
                sl = slice(kb * 512, (kb + 1) * 512)
                xw = xwp.tile([128, NCH, 512], f32r, tag="xw")
                for c in range(NCH):
                    nc.sync.dma_start(
                        out=xw[:, c, :],
                        in_=XTKV[c * 128:(c + 1) * 128, sl].bitcast(f32r))
                pk = pp.tile([128, 512], f32, tag="pq")
                for c in range(NCH):
                    nc.tensor.matmul(pk, wk[:, c, :], xw[:, c, :],
                                     start=(c == 0), stop=(c == NCH - 1))
                nc.vector.tensor_scalar_add(kT[:, sl], pk, bk)
                pv = pp.tile([128, 512], f32, tag="pq")
                for c in range(NCH):
                    nc.tensor.matmul(pv, wv[:, c, :], xw[:, c, :],
                                     start=(c == 0), stop=(c == NCH - 1))
                nc.vector.tensor_scalar_add(vT[:, sl], pv, bv)

        # ---- v1 = vT transposed into [k, d] chunks (+ ones col via memset)
        with tc.tile_pool(name="ptp", bufs=3, space="PSUM") as ptp:
            for kc in range(nkca):
                pt = ptp.tile([128, 128], f32, tag="pt")
                nc.tensor.transpose(pt, vT[:, kc * 128:(kc + 1) * 128], ident)
                nc.vector.tensor_copy(v1[:, 0, kc, 0:64], pt[:, 0:64])
                nc.vector.tensor_copy(v1[:, 1, kc, 0:64], pt[:, 64:128])

        # ---- attention (flash-style, scores transposed [k, q]).
        # Both heads' score matmuls are emitted back-to-back: K=64 each at
        # PE row-groups 0-63 / 64-127 (derived from base_partition), so they
        # execute concurrently in the array. One Exp per chunk covers both
        # heads' scores (1024 elems), amortizing ACT overhead.
        with tc.tile_pool(name="pss", bufs=2, space="PSUM") as pss, \
             tc.tile_pool(name="psc", bufs=1, space="PSUM") as psc, \
             tc.tile_pool(name="pst", bufs=2, space="PSUM") as pst, \
             tc.tile_pool(name="ep", bufs=3) as ep, \
             tc.tile_pool(name="cp", bufs=4) as cp, \
             tc.tile_pool(name="op", bufs=3) as op, \
             tc.tile_pool(name="lp", bufs=6) as lp:
            h0 = slice(0, 64)
            h1 = slice(64, 128)
            for qt in range(NQT):
                qsl = slice(qt * 512, (qt + 1) * 512)
                pc0 = psc.tile([65, 512], f32, tag="pc0")
                pc1 = psc.tile([65, 512], f32, tag="pc1")
                prev = None  # (chunk, eT tile)
                for kc in range(nkca):
                    ksl = slice(kc * 128, (kc + 1) * 128)
                    ss = pss.tile([128, 2, 512], f32, tag="ss")
                    nc.tensor.matmul(ss[:, 0, :], kT[h0, ksl], qT[h0, qsl],
                                     start=True, stop=True)
                    nc.tensor.matmul(ss[:, 1, :], kT[h1, ksl], qT[h1, qsl],
                                     start=True, stop=True)
                    et = ep.tile([128, 2, 512], f32r, tag="et")
                    bias = mb[:, nkca - 1:nkca] if kc == nkca - 1 else 0.0
                    nc.scalar.activation(et, ss, AF.Exp, bias=bias, scale=SCALE)
                    if prev is not None:
                        pkc, pet = prev
                        nc.tensor.matmul(pc0, v1[:, 0, pkc, :], pet[:, 0, :],
                                         start=(pkc == 0), stop=False)
                        nc.tensor.matmul(pc1, v1[:, 1, pkc, :], pet[:, 1, :],
                                         start=(pkc == 0), stop=False)
                    prev = (kc, et)
                pkc, pet = prev
                nc.tensor.matmul(pc0, v1[:, 0, pkc, :], pet[:, 0, :],
                                 start=(pkc == 0), stop=True)
                nc.tensor.matmul(pc1, v1[:, 1, pkc, :], pet[:, 1, :],
                                 start=(pkc == 0), stop=True)
                cts = []
                for h, pc in ((0, pc0), (1, pc1)):
                    ct = cp.tile([65, 512], f32, tag="ct")
                    nc.vector.tensor_copy(ct, pc)
                    cts.append(ct)
                for qb in range(4):
                    pt2 = pst.tile([128, 130], f32, tag="pt2")
                    bsl = slice(qb * 128, (qb + 1) * 128)
                    nc.tensor.transpose(pt2[:, 0:65], cts[0][:, bsl], ident[0:65, 0:65])
                    nc.tensor.transpose(pt2[:, 65:130], cts[1][:, bsl], ident[0:65, 0:65])
                    li0 = lp.tile([128, 1], f32, tag="li")
                    li1 = lp.tile([128, 1], f32, tag="li")
                    nc.vector.reciprocal(li0, pt2[:, 64:65])
                    nc.vector.reciprocal(li1, pt2[:, 129:130])
                    ob = op.tile([128, 128], f32, tag="ob")
                    nc.vector.tensor_scalar_mul(ob[:, 0:64], pt2[:, 0:64], li0)
                    nc.vector.tensor_scalar_mul(ob[:, 64:128], pt2[:, 65:129], li1)
                    r = qt * 4 + qb
                    nc.sync.dma_start(out=OUT[r * 128:(r + 1) * 128, :], in_=ob)


_NC = {}


def _build(nkb, nkca):
    key = (nkb, nkca)
    if key in _NC:
        return _NC[key]
    nc = bacc.Bacc("TRN2", target_bir_lowering=False, debug=False)
    skp = 512 * nkb
    XT = nc.dram_tensor("XT", [HID, S], f32, kind="ExternalInput").ap()
    XTKV = nc.dram_tensor("XTKV", [HID, skp], f32, kind="ExternalInput").ap()
    WQ = nc.dram_tensor("WQ", [HID, D2], f32, kind="ExternalInput").ap()
    WK = nc.dram_tensor("WK", [HID, D2], f32, kind="ExternalInput").ap()
    WV = nc.dram_tensor("WV", [HID, D2], f32, kind="ExternalInput").ap()
    BQ = nc.dram_tensor("BQ", [D2], f32, kind="ExternalInput").ap()
    BK = nc.dram_tensor("BK", [D2], f32, kind="ExternalInput").ap()
    BV = nc.dram_tensor("BV", [D2], f32, kind="ExternalInput").ap()
    MB = nc.dram_tensor("MB", [128, nkca], f32, kind="ExternalInput").ap()
    OUT = nc.dram_tensor("OUT", [S, D2], f32, kind="ExternalOutput").ap()
    with tile.TileContext(nc) as tc:
        _emit(nc, tc, (XT, XTKV, WQ, WK, WV, BQ, BK, BV, MB, OUT), nkb, nkca)
    nc.compile()
    _NC[key] = nc
    return nc


def make_in_maps(hidden_states, attention_mask, Wq, bq, Wk, bk, Wv, bv):
    x = np.asarray(hidden_states, dtype=np.float32).reshape(S, HID)
    xT = np.ascontiguousarray(x.T)
    mask = np.asarray(attention_mask).reshape(S).astype(bool)
    idx = np.nonzero(mask)[0]
    m = len(idx)
    nkca = max(1, (m + 127) // 128)
    nkb = max(1, (nkca * 128 + 511) // 512)
    skp = nkb * 512
    # pad with position 0 (values are finite; pad slots masked to -inf below)
    idx_p = np.zeros(skp, np.int64)
    idx_p[:m] = idx
    xTkv = np.ascontiguousarray(xT[:, idx_p])
    mbias = np.full(nkca * 128, np.float32(NEG), np.float32)
    mbias[:m] = 0.0
    MBn = np.ascontiguousarray(mbias.reshape(nkca, 128).T)
    Wq = np.asarray(Wq, np.float32)
    Wk = np.asarray(Wk, np.float32)
    Wv = np.asarray(Wv, np.float32)
    bq = np.asarray(bq, np.float32)
    bk = np.asarray(bk, np.float32)
    bv = np.asarray(bv, np.float32)
    in_maps = []
    for c in range(8):
        sl = slice(D2 * c, D2 * (c + 1))
        in_maps.append({
            "XT": xT, "XTKV": xTkv, "MB": MBn,
            "WQ": np.ascontiguousarray(Wq[:, sl]),
            "WK": np.ascontiguousarray(Wk[:, sl]),
            "WV": np.ascontiguousarray(Wv[:, sl]),
            "BQ": np.ascontiguousarray(bq[sl]),
            "BK": np.ascontiguousarray(bk[sl]),
            "BV": np.ascontiguousarray(bv[sl]),
        })
    return in_maps, nkb, nkca


def kernel(hidden_states, attention_mask, Wq, bq, Wk, bk, Wv, bv):
    in_maps, nkb, nkca = make_in_maps(
        hidden_states, attention_mask, Wq, bq, Wk, bk, Wv, bv)
    nc = _build(nkb, nkca)
    res = run_bass_kernel_spmd(nc, in_maps, list(range(8)))
    out = np.concatenate([res.results[c]["OUT"] for c in range(8)], axis=1)
    return (out.reshape(1, S, HID),)


# revision 9
# speedup vs baseline: 3.4936x; 1.1441x over previous
"""Bass/Trainium2 kernel for BERT-style masked attention (B=1, S=4096, HID=1024, H=16).

Strategy: tensor-parallel over heads across 8 NeuronCores (2 heads/core).
Each core computes q/k/v projections for its 128 output columns from the
full (host-pretransposed) hidden states, runs masked softmax attention for
its 2 heads fully on-chip (flash-style, scores never hit DRAM), and writes
its [4096, 128] slice of the context. Host concatenates slices.

The key mask is key-only (same for every query/head), so masked key
positions are compacted away host-side: k/v projections and the attention
inner loop run only over the ~(S/2) surviving key positions.
"""

import numpy as np
from contextlib import ExitStack

import concourse.bass as bass
import concourse.tile as tile
from concourse import bacc, mybir
from concourse.bass_utils import run_bass_kernel_spmd
from concourse.masks import make_identity

f32 = mybir.dt.float32
f32r = mybir.dt.float32r
AF = mybir.ActivationFunctionType

S = 4096
HID = 1024
D2 = 128          # per-core output columns (2 heads x 64)
NCH = HID // 128  # 8 hid chunks
NSB = S // 512    # 8 sequence blocks for q projections
NQT = S // 512    # 8 query tiles
SCALE = 64 ** -0.5
NEG = -1e30
GRP = 2           # key chunks per attention group


def _emit(nc, tc, aps, nkb, nkca):
    """nkb: # 512-wide key blocks for k/v projections (SKP = 512*nkb).
    nkca: # 128-wide key chunks the attention loop visits (<= 4*nkb)."""
    XT, XTKV, WQ, WK, WV, BQ, BK, BV, MB, OUT = aps
    skp = 512 * nkb
    with ExitStack() as top:
        const = top.enter_context(tc.tile_pool(name="const", bufs=1))
        big = top.enter_context(tc.tile_pool(name="big", bufs=1))

        ident = const.tile([128, 128], f32)
        make_identity(nc, ident)

        wq = const.tile([128, NCH, 128], f32r)
        wk = const.tile([128, NCH, 128], f32r)
        wv = const.tile([128, NCH, 128], f32r)
        nc.sync.dma_start(out=wk, in_=WK.rearrange("(c p) d -> p c d", p=128).bitcast(f32r))
        nc.sync.dma_start(out=wv, in_=WV.rearrange("(c p) d -> p c d", p=128).bitcast(f32r))
        nc.sync.dma_start(out=wq, in_=WQ.rearrange("(c p) d -> p c d", p=128).bitcast(f32r))

        bq = const.tile([128, 1], f32)
        bk = const.tile([128, 1], f32)
        bv = const.tile([128, 1], f32)
        nc.gpsimd.dma_start(out=bq, in_=BQ.unsqueeze(1))
        nc.gpsimd.dma_start(out=bk, in_=BK.unsqueeze(1))
        nc.gpsimd.dma_start(out=bv, in_=BV.unsqueeze(1))

        mb = const.tile([128, nkca], f32)
        nc.gpsimd.dma_start(out=mb, in_=MB)

        qT = big.tile([128, S], f32r)     # [d2, s] queries (both heads stacked)
        kT = big.tile([128, skp], f32r)   # [d2, sk] keys (compacted)
        vT = big.tile([128, skp], f32)    # [d2, sk] values (pre-transpose)
        v1 = big.tile([128, 2, nkca, 65], f32r)  # [k, head, chunk, d|1]
        ones_c = const.tile([128, 1], f32)
        nc.vector.memset(ones_c, 1.0)
        nc.vector.tensor_copy(v1[:, 0, :, 64:65], ones_c.to_broadcast((128, nkca, 1)))
        nc.vector.tensor_copy(v1[:, 1, :, 64:65], ones_c.to_broadcast((128, nkca, 1)))

        # ---- k/v projections (accumulate over hid chunks, f32r full-rate)
        with tc.tile_pool(name="xwkp", bufs=3) as xwkp, \
             tc.tile_pool(name="pp", bufs=2, space="PSUM") as pp:
            for kb in range(nkb):
                sl = slice(kb * 512, (kb + 1) * 512)
                xw = xwkp.tile([128, NCH, 512], f32r, tag="xwk")
                for c in range(NCH):
                    nc.sync.dma_start(
                        out=xw[:, c, :],
                        in_=XTKV[c * 128:(c + 1) * 128, sl].bitcast(f32r))
                pk = pp.tile([128, 512], f32, tag="pq")
                for c in range(NCH):
                    nc.tensor.matmul(pk, wk[:, c, :], xw[:, c, :],
                                     start=(c == 0), stop=(c == NCH - 1))
                nc.vector.tensor_scalar_add(kT[:, sl], pk, bk)
                pv = pp.tile([128, 512], f32, tag="pq")
                for c in range(NCH):
                    nc.tensor.matmul(pv, wv[:, c, :], xw[:, c, :],
                                     start=(c == 0), stop=(c == NCH - 1))
                nc.vector.tensor_scalar_add(vT[:, sl], pv, bv)

        # ---- v1 = vT transposed into [k, d] chunks (+ ones col)
        with tc.tile_pool(name="ptp", bufs=3, space="PSUM") as ptp:
            for kc in range(nkca):
                pt = ptp.tile([128, 128], f32, tag="pt")
                nc.tensor.transpose(pt, vT[:, kc * 128:(kc + 1) * 128], ident)
                nc.vector.tensor_copy(v1[:, 0, kc, 0:64], pt[:, 0:64])
                nc.vector.tensor_copy(v1[:, 1, kc, 0:64], pt[:, 64:128])

        # ---- attention (flash-style, scores transposed [k, q]).
        # Both heads' score matmuls are emitted back-to-back: K=64 each at
        # PE row-groups 0-63 / 64-127 (derived from base_partition), so they
        # execute concurrently in the array. One Exp per chunk covers both
        # heads' scores (1024 elems), amortizing ACT overhead.
        with tc.tile_pool(name="xwp", bufs=3) as xwp, \
             tc.tile_pool(name="ppq", bufs=1, space="PSUM") as ppq, \
             tc.tile_pool(name="pss", bufs=2, space="PSUM") as pss, \
             tc.tile_pool(name="psc", bufs=1, space="PSUM") as psc, \
             tc.tile_pool(name="pst", bufs=1, space="PSUM") as pst, \
             tc.tile_pool(name="ep", bufs=3) as ep, \
             tc.tile_pool(name="cp", bufs=4) as cp, \
             tc.tile_pool(name="op", bufs=3) as op, \
             tc.tile_pool(name="lp", bufs=6) as lp:
            h0 = slice(0, 64)
            h1 = slice(64, 128)
            for qt in range(NQT):
                qsl = slice(qt * 512, (qt + 1) * 512)
                # q projection for this query tile (overlaps attention)
                xw = xwp.tile([128, NCH, 512], f32r, tag="xw")
                for c in range(NCH):
                    nc.sync.dma_start(
                        out=xw[:, c, :],
                        in_=XT[c * 128:(c + 1) * 128, qsl].bitcast(f32r))
                pq = ppq.tile([128, 512], f32, tag="pq2")
                for c in range(NCH):
                    nc.tensor.matmul(pq, wq[:, c, :], xw[:, c, :],
                                     start=(c == 0), stop=(c == NCH - 1))
                nc.vector.tensor_scalar_add(qT[:, qsl], pq, bq)
                pc0 = psc.tile([65, 512], f32, tag="pc0")
                pc1 = psc.tile([65, 512], f32, tag="pc1")
                prev = None  # (chunk, eT tile)
                for kc in range(nkca):
                    ksl = slice(kc * 128, (kc + 1) * 128)
                    ss = pss.tile([128, 2, 512], f32, tag="ss")
                    nc.tensor.matmul(ss[:, 0, :], kT[h0, ksl], qT[h0, qsl],
                                     start=True, stop=True)
                    nc.tensor.matmul(ss[:, 1, :], kT[h1, ksl], qT[h1, qsl],
                                     start=True, stop=True)
                    et = ep.tile([128, 2, 512], f32r, tag="et")
                    bias = mb[:, nkca - 1:nkca] if kc == nkca - 1 else 0.0
                    nc.scalar.activation(et, ss, AF.Exp, bias=bias, scale=SCALE)
                    if prev is not None:
                        pkc, pet = prev
                        nc.tensor.matmul(pc0, v1[:, 0, pkc, :], pet[:, 0, :],
                                         start=(pkc == 0), stop=False)
                        nc.tensor.matmul(pc1, v1[:, 1, pkc, :], pet[:, 1, :],
                                         start=(pkc == 0), stop=False)
                    prev = (kc, et)
                pkc, pet = prev
                nc.tensor.matmul(pc0, v1[:, 0, pkc, :], pet[:, 0, :],
                                 start=(pkc == 0), stop=True)
                nc.tensor.matmul(pc1, v1[:, 1, pkc, :], pet[:, 1, :],
                                 start=(pkc == 0), stop=True)
                cts = []
                for h, pc in ((0, pc0), (1, pc1)):
                    ct = cp.tile([65, 512], f32, tag="ct")
                    nc.vector.tensor_copy(ct, pc)
                    cts.append(ct)
                for qb in range(4):
                    pt2 = pst.tile([128, 130], f32, tag="pt2")
                    bsl = slice(qb * 128, (qb + 1) * 128)
                    nc.tensor.transpose(pt2[:, 0:65], cts[0][:, bsl], ident[0:65, 0:65])
                    nc.tensor.transpose(pt2[:, 65:130], cts[1][:, bsl], ident[0:65, 0:65])
                    li0 = lp.tile([128, 1], f32, tag="li")
                    li1 = lp.tile([128, 1], f32, tag="li")
                    nc.vector.reciprocal(li0, pt2[:, 64:65])
                    nc.vector.reciprocal(li1, pt2[:, 129:130])
                    ob = op.tile([128, 128], f32, tag="ob")
                    nc.vector.tensor_scalar_mul(ob[:, 0:64], pt2[:, 0:64], li0)
                    nc.vector.tensor_scalar_mul(ob[:, 64:128], pt2[:, 65:129], li1)
                    r = qt * 4 + qb
                    nc.sync.dma_start(out=OUT[r * 128:(r + 1) * 128, :], in_=ob)


_NC = {}


def _build(nkb, nkca):
    key = (nkb, nkca)
    if key in _NC:
        return _NC[key]
    nc = bacc.Bacc("TRN2", target_bir_lowering=False, debug=False)
    skp = 512 * nkb
    XT = nc.dram_tensor("XT", [HID, S], f32, kind="ExternalInput").ap()
    XTKV = nc.dram_tensor("XTKV", [HID, skp], f32, kind="ExternalInput").ap()
    WQ = nc.dram_tensor("WQ", [HID, D2], f32, kind="ExternalInput").ap()
    WK = nc.dram_tensor("WK", [HID, D2], f32, kind="ExternalInput").ap()
    WV = nc.dram_tensor("WV", [HID, D2], f32, kind="ExternalInput").ap()
    BQ = nc.dram_tensor("BQ", [D2], f32, kind="ExternalInput").ap()
    BK = nc.dram_tensor("BK", [D2], f32, kind="ExternalInput").ap()
    BV = nc.dram_tensor("BV", [D2], f32, kind="ExternalInput").ap()
    MB = nc.dram_tensor("MB", [128, nkca], f32, kind="ExternalInput").ap()
    OUT = nc.dram_tensor("OUT", [S, D2], f32, kind="ExternalOutput").ap()
    with tile.TileContext(nc) as tc:
        _emit(nc, tc, (XT, XTKV, WQ, WK, WV, BQ, BK, BV, MB, OUT), nkb, nkca)
    nc.compile()
    _NC[key] = nc
    return nc


def make_in_maps(hidden_states, attention_mask, Wq, bq, Wk, bk, Wv, bv):
    x = np.asarray(hidden_states, dtype=np.float32).reshape(S, HID)
    xT = np.ascontiguousarray(x.T)
    mask = np.asarray(attention_mask).reshape(S).astype(bool)
    idx = np.nonzero(mask)[0]
    m = len(idx)
    nkca = max(1, (m + 127) // 128)
    nkb = max(1, (nkca * 128 + 511) // 512)
    skp = nkb * 512
    # pad with position 0 (values are finite; pad slots masked to -inf below)
    idx_p = np.zeros(skp, np.int64)
    idx_p[:m] = idx
    xTkv = np.ascontiguousarray(xT[:, idx_p])
    mbias = np.full(nkca * 128, np.float32(NEG), np.float32)
    mbias[:m] = 0.0
    MBn = np.ascontiguousarray(mbias.reshape(nkca, 128).T)
    Wq = np.asarray(Wq, np.float32)
    Wk = np.asarray(Wk, np.float32)
    Wv = np.asarray(Wv, np.float32)
    bq = np.asarray(bq, np.float32)
    bk = np.asarray(bk, np.float32)
    bv = np.asarray(bv, np.float32)
    in_maps = []
    for c in range(8):
        sl = slice(D2 * c, D2 * (c + 1))
        in_maps.append({
            "XT": xT, "XTKV": xTkv, "MB": MBn,
            "WQ": np.ascontiguousarray(Wq[:, sl]),
            "WK": np.ascontiguousarray(Wk[:, sl]),
            "WV": np.ascontiguousarray(Wv[:, sl]),
            "BQ": np.ascontiguousarray(bq[sl]),
            "BK": np.ascontiguousarray(bk[sl]),
            "BV": np.ascontiguousarray(bv[sl]),
        })
    return in_maps, nkb, nkca


def kernel(hidden_states, attention_mask, Wq, bq, Wk, bk, Wv, bv):
    in_maps, nkb, nkca = make_in_maps(
        hidden_states, attention_mask, Wq, bq, Wk, bk, Wv, bv)
    nc = _build(nkb, nkca)
    res = run_bass_kernel_spmd(nc, in_maps, list(range(8)))
    out = np.concatenate([res.results[c]["OUT"] for c in range(8)], axis=1)
    return (out.reshape(1, S, HID),)
